# revision 20
# baseline (speedup 1.0000x reference)
"""Trainium2 Bass kernel for a cross-attention transformer block.

Sharding: 8 cores = 4 batch rows x 2 query-groups of 512 (even/odd row
interleave, so every core sees a near-identical time-mask profile and the
SPMD-shared span plan is tight).

On-device per core: modality-embedding gather (indirect DMA, host-computed
row indices), layernorms (bn_stats + PE-transpose, LN scale/bias fused into
the ACT psum->sbuf copy, outputs quantized to fp8e4 where consumed by fp8
matmuls), Q/K/V and c-proj as fp8e4 DoubleRow matmuls (2 contraction
chunks per instruction), attention with per-key-chunk query spans computed
from the actual mask (exact for arbitrary inputs), masked softmax as
exp(S)*mask with a ones-row in V providing denominators and a rank-1
eps matmul guaranteeing den>0 and PSUM coverage, softmax division via
reciprocal + gpsimd partition_broadcast + vector multiply, MLP with an
fp8e4 DoubleRow fc (streamed in 8-wide f-groups) and a bf16 proj with
resident weights and per-q-tile streamed output. fp8 weights are
pre-scaled by 16 on host with the unscale folded into the psum->sbuf
copies; the attention softmax scale is folded into the Q copy.

Host computes gather indices and the span plan (ints only); all float math
runs on device.
"""

import numpy as np
import ml_dtypes

import concourse.bass as bass
import concourse.tile as tile
from concourse import bacc, mybir
from concourse.bass_utils import run_bass_kernel_spmd
from concourse.masks import make_identity

dt = mybir.dt
AF = mybir.ActivationFunctionType
ALU = mybir.AluOpType
PM = mybir.MatmulPerfMode

B = 4
LQ = 512          # queries per core
LK = 1024         # side (key) sequence length
E = 768
H = 12
DH = 64
F = 3072
HALF = 512
EC = E // 128     # 6
FC = F // 128     # 24
KC = LK // 128    # 8
QC = LQ // 128    # 4
LN_EPS = 1e-5
N_CORES = 8
BF = ml_dtypes.bfloat16
F8 = ml_dtypes.float8_e4m3
WS = 16.0         # fp8 weight pre-scale
DEN_EPS = 1e-6

_prog_cache = {}

DEBUG_NAMES = ()


def _build_program(entries, ln_ident, zb):
    """entries: (kc, q0, q1) span rows; ln_ident: 3 bools; zb: vb/cb/pb==0."""
    nc = bacc.Bacc("TRN2", target_bir_lowering=False, debug=False,
                   num_devices=N_CORES)

    def din(name, shape, dty=dt.float32):
        return nc.dram_tensor(name, shape, dty, kind="ExternalInput").ap()

    xh = din("xh", [LQ, E])
    embcat = din("embcat", [2 * HALF + 1, E], dt.bfloat16)
    gidx = din("gidx", [LK], dt.int32)
    ageh = din("ageh", [LQ])
    modage = din("modage", [LK])
    qw8 = din("qw8", [128, 3, 2, E], dt.float8e4)
    kw8 = din("kw8", [128, 3, 2, E], dt.float8e4)
    vw8 = din("vw8", [128, 3, 2, E], dt.float8e4)
    cw8 = din("cw8", [128, 3, 2, E], dt.float8e4)
    fcw = din("fcw", [128, FC // 8, 8, 3, 2, 128], dt.float8e4)
    pw = din("pw", [128, FC, E], dt.bfloat16)
    blob = din("blob", [128, 72 + 3 * E])

    out = nc.dram_tensor("out", [LQ, E], dt.float32, kind="ExternalOutput").ap()
    dbg = {}
    def dout(name, shape, dty=dt.bfloat16):
        dbg[name] = nc.dram_tensor("dbg_" + name, shape, dty,
                                   kind="ExternalOutput").ap()
    for nm in DEBUG_NAMES:
        if nm == "QT": dout(nm, [128, EC, LQ])
        if nm == "KT": dout(nm, [128, EC, LK])
        if nm == "Vaug": dout(nm, [128, KC, H * (DH + 1)])
        if nm == "Yall": dout(nm, [128, EC, LQ])
        if nm == "x2": dout(nm, [128, QC, E], dt.float32)

    nE = len(entries)

    with tile.TileContext(nc) as tc:
        # LIFO pool stack (opened in reverse release order)
        singles = tc.alloc_tile_pool(name="singles", bufs=1)
        pool_pw = tc.alloc_tile_pool(name="pool_pw", bufs=1)
        pool_mlp = tc.alloc_tile_pool(name="pool_mlp", bufs=1)
        pool_x2 = tc.alloc_tile_pool(name="pool_x2", bufs=1)
        pool_w = tc.alloc_tile_pool(name="pool_w", bufs=1)
        pool_yall = tc.alloc_tile_pool(name="pool_yall", bufs=1)
        pool_att = tc.alloc_tile_pool(name="pool_att", bufs=1)
        pool_sn = tc.alloc_tile_pool(name="pool_sn", bufs=1)
        pool_xc = tc.alloc_tile_pool(name="pool_xc", bufs=1)
        pool_mx = tc.alloc_tile_pool(name="pool_mx", bufs=1)

        # -------- input DMAs (sync queue, urgent first) --------
        gidx_t = singles.tile([128, KC], dt.int32)
        nc.sync.dma_start(gidx_t[:], gidx.rearrange("(c p) -> p c", p=128))
        modage_t = singles.tile([128, KC], dt.float32)
        nc.sync.dma_start(modage_t[:], modage.rearrange("(c p) -> p c", p=128))
        vw8_t = pool_w.tile([128, 3, 2, E], dt.float8e4, tag="vw8")
        nc.sync.dma_start(vw8_t[:], vw8)
        xc = pool_xc.tile([128, QC, E], dt.float32)
        nc.sync.dma_start(xc[:], xh.rearrange("(c p) e -> p c e", p=128))
        kw8_t = pool_w.tile([128, 3, 2, E], dt.float8e4, tag="kw8")
        nc.sync.dma_start(kw8_t[:], kw8)
        qw8_t = pool_w.tile([128, 3, 2, E], dt.float8e4, tag="qw8")
        nc.sync.dma_start(qw8_t[:], qw8)
        blob_t = singles.tile([128, 72 + 3 * E], dt.float32)
        nc.sync.dma_start(blob_t[:], blob)

        # gather on gpsimd queue (pipelines with LN0 per chunk)
        mxall = pool_mx.tile([128, KC, E], dt.bfloat16)
        for kc in range(KC):
            nc.gpsimd.indirect_dma_start(
                out=mxall[:, kc, :], out_offset=None, in_=embcat,
                in_offset=bass.IndirectOffsetOnAxis(
                    ap=gidx_t[:, kc:kc + 1], axis=0))
        age_bc = singles.tile([128, LQ], dt.float32)
        nc.sync.dma_start(age_bc[:], bass.AP(
            tensor=ageh.tensor, offset=ageh.offset, ap=[[0, 128]] + ageh.ap))
        cw8_t = pool_w.tile([128, 3, 2, E], dt.float8e4, tag="cw8")
        nc.sync.dma_start(cw8_t[:], cw8)


        # blob column views
        qb_c = blob_t[:, 0:6]
        kb_c = blob_t[:, 6:12]
        fcb_c = blob_t[:, 12:36]
        ln0w_c = blob_t[:, 36:42]; ln0b_c = blob_t[:, 42:48]
        ln1w_c = blob_t[:, 48:54]; ln1b_c = blob_t[:, 54:60]
        ln2w_c = blob_t[:, 60:66]; ln2b_c = blob_t[:, 66:72]
        vb_r = blob_t[:, 72:72 + E]
        cb_r = blob_t[:, 72 + E:72 + 2 * E]
        pb_r = blob_t[:, 72 + 2 * E:72 + 3 * E]

        ident = singles.tile([128, 128], dt.bfloat16)
        make_identity(nc, ident[:])
        eps_t = singles.tile([128, 1], dt.float32)
        nc.vector.memset(eps_t[:], LN_EPS)
        ones1p = singles.tile([1, LQ], dt.bfloat16)
        nc.vector.memset(ones1p[:], 1.0)
        epsc = singles.tile([1, DH + 1], dt.bfloat16)
        nc.vector.memset(epsc[:], 0.0)
        nc.vector.memset(epsc[:, DH:DH + 1], DEN_EPS)
        nonneg = singles.tile([128, KC], dt.float32)
        nc.vector.tensor_scalar(out=nonneg[:], in0=modage_t[:], scalar1=0.0,
                                scalar2=None, op0=ALU.is_ge)

        # -------- layernorm -> transposed output --------
        def make_ln(src_tile, n_chunks, lnw, lnb, identity_ln, dstT,
                    ln_pool, ps_pool, tagp, dst_col0=0, xhat_scalar=False):
            """Returns emit_chunk(c): full LN pipeline for one 128-token
            chunk; stats on vector, sqrt on scalar, xhat on vector (or
            scalar via -m*rstd bias), PE transposes, psum->sbuf copies on
            scalar."""
            mvs = ln_pool.tile([128, n_chunks, 2], dt.float32, tag=tagp + "mvs")
            rstd = ln_pool.tile([128, n_chunks], dt.float32, tag=tagp + "rst")
            nmr = ln_pool.tile([128, n_chunks], dt.float32, tag=tagp + "nmr")

            def emit_chunk(c):
                stats = ln_pool.tile([128, 2, 6], dt.float32, tag="ln_st")
                nc.vector.bn_stats(out=stats[:, 0, :],
                                   in_=src_tile[:, c, 0:384])
                nc.vector.bn_stats(out=stats[:, 1, :],
                                   in_=src_tile[:, c, 384:768])
                nc.vector.bn_aggr(out=mvs[:, c, :], in_=stats[:])
                nc.scalar.activation(out=rstd[:, c:c + 1], in_=mvs[:, c, 1:2],
                                     func=AF.Sqrt, bias=eps_t[:], scale=1.0)
                nc.vector.reciprocal_approx_fast(out=rstd[:, c:c + 1],
                                                 in_=rstd[:, c:c + 1])
                xhat = ln_pool.tile([128, E], dt.bfloat16, tag="ln_xhat")
                if xhat_scalar:
                    nc.vector.tensor_scalar(out=nmr[:, c:c + 1],
                                            in0=mvs[:, c, 0:1], scalar1=-1.0,
                                            scalar2=rstd[:, c:c + 1],
                                            op0=ALU.mult, op1=ALU.mult)
                    nc.scalar.activation(out=xhat[:], in_=src_tile[:, c, :],
                                         func=AF.Identity,
                                         bias=nmr[:, c:c + 1],
                                         scale=rstd[:, c:c + 1])
                else:
                    nc.vector.tensor_scalar(out=xhat[:], in0=src_tile[:, c, :],
                                            scalar1=mvs[:, c, 0:1],
                                            scalar2=rstd[:, c:c + 1],
                                            op0=ALU.subtract, op1=ALU.mult)
                col = dst_col0 + c * 128
                if identity_ln:
                    for half in range(2):
                        pt = ps_pool.tile([128, 3, 128], dt.bfloat16,
                                          space="PSUM", tag="ln_tp3")
                        for j in range(3):
                            ec = half * 3 + j
                            nc.tensor.transpose(
                                pt[:, j, :], xhat[:, ec * 128:(ec + 1) * 128],
                                ident[:])
                        nc.scalar.activation(
                            out=dstT[:, half * 3:(half + 1) * 3, col:col + 128],
                            in_=pt[:], func=AF.Identity, bias=0.0, scale=1.0)
                else:
                    for ec in range(EC):
                        pt = ps_pool.tile([128, 128], dt.bfloat16,
                                          space="PSUM", tag="ln_tp")
                        nc.tensor.transpose(
                            pt[:], xhat[:, ec * 128:(ec + 1) * 128], ident[:])
                        nc.scalar.activation(
                            out=dstT[:, ec, col:col + 128],
                            in_=pt[:], func=AF.Identity,
                            bias=lnb[:, ec:ec + 1], scale=lnw[:, ec:ec + 1])

            return emit_chunk

        # ======== LN0 -> snT8; V per chunk; K; then LN1 -> xnT8, Q ========
        QT = pool_att.tile([128, EC, LQ], dt.bfloat16, tag="QT")
        xnT8 = pool_sn.tile([128, EC, LQ], dt.float8e4, tag="xnT8")
        snT8 = pool_sn.tile([128, EC, LK], dt.float8e4, tag="snT8")
        KT = pool_att.tile([128, EC, LK], dt.bfloat16, tag="KT")
        Vaug = pool_att.tile([128, KC, H * (DH + 1)], dt.bfloat16, tag="Vaug")
        nc.vector.memset(
            Vaug[:].rearrange("p c (h x) -> p c h x", x=DH + 1)[:, :, :, DH:DH + 1],
            1.0)

        def emit_V(kc, phA_pv):
            for hf in range(2):
                pv = phA_pv.tile([128, 384], dt.float32, space="PSUM",
                                 tag="pv")
                for j in range(3):
                    nc.tensor.matmul(
                        pv[:], snT8[:, 2 * j:2 * j + 2, kc * 128:(kc + 1) * 128],
                        vw8_t[:, j, :, hf * 384:(hf + 1) * 384],
                        start=(j == 0), stop=(j == 2), perf_mode=PM.DoubleRow)
                dstv = Vaug[:, kc, :].rearrange(
                    "p (h x) -> p h x", x=DH + 1)[:, hf * 6:(hf + 1) * 6, 0:DH]
                pvh = pv[:].rearrange("p (h x) -> p h x", x=DH)
                if zb[0]:
                    nc.scalar.activation(out=dstv, in_=pvh, func=AF.Copy,
                                         bias=0.0, scale=1.0 / WS)
                else:
                    nc.vector.scalar_tensor_tensor(
                        out=dstv, in0=pvh, scalar=1.0 / WS,
                        in1=vb_r[:, hf * 384:(hf + 1) * 384].rearrange(
                            "p (h x) -> p h x", x=DH),
                        op0=ALU.mult, op1=ALU.add)

        def emit_K(ks, phA_pv):
            for dc in range(EC):
                pk = phA_pv.tile([128, 512], dt.float32, space="PSUM",
                                 tag="pk")
                for j in range(3):
                    nc.tensor.matmul(
                        pk[:], kw8_t[:, j, :, dc * 128:(dc + 1) * 128],
                        snT8[:, 2 * j:2 * j + 2, ks * 512:(ks + 1) * 512],
                        start=(j == 0), stop=(j == 2), perf_mode=PM.DoubleRow)
                nc.scalar.activation(
                    out=KT[:, dc, ks * 512:(ks + 1) * 512], in_=pk[:],
                    func=AF.Identity, bias=kb_c[:, dc:dc + 1], scale=1.0 / WS)

        with tc.tile_pool(name="phA", bufs=2) as phA, \
             tc.tile_pool(name="phA_ps", bufs=3, space="PSUM") as phA_ps, \
             tc.tile_pool(name="phA_pv", bufs=2, space="PSUM") as phA_pv:
            ln0 = make_ln(mxall[:], KC, ln0w_c, ln0b_c, ln_ident[0], snT8,
                          phA, phA_ps, "l0")
            ln1 = make_ln(xc[:], QC, ln1w_c, ln1b_c, ln_ident[1], xnT8,
                          phA, phA_ps, "l1")
            for c in range(KC):
                ln0(c)
                emit_V(c, phA_pv)
                if c % 2 == 1:
                    ln1(c // 2)
                if c == 3:
                    emit_K(0, phA_pv)
                if c == KC - 1:
                    emit_K(1, phA_pv)
            for dc in range(EC):
                pq = phA_pv.tile([128, LQ], dt.float32, space="PSUM", tag="pk")
                for j in range(3):
                    nc.tensor.matmul(
                        pq[:], qw8_t[:, j, :, dc * 128:(dc + 1) * 128],
                        xnT8[:, 2 * j:2 * j + 2, :],
                        start=(j == 0), stop=(j == 2), perf_mode=PM.DoubleRow)
                nc.scalar.activation(out=QT[:, dc, :], in_=pq[:],
                                     func=AF.Identity,
                                     bias=qb_c[:, dc:dc + 1], scale=1.0 / (WS * 8.0))
        pool_mx.release()
        pool_xc.release()
        pool_sn.release()
        if "QT" in dbg: nc.sync.dma_start(dbg["QT"], QT[:])
        if "KT" in dbg: nc.sync.dma_start(dbg["KT"], KT[:])
        if "Vaug" in dbg: nc.sync.dma_start(dbg["Vaug"], Vaug[:])

        # -------- mask build (vector) --------
        mask = pool_att.tile([128, KC, LQ], dt.bfloat16, tag="mask")
        for kc, q0, q1 in entries:
            if q1 > q0:
                nc.vector.tensor_scalar(out=mask[:, kc, q0:q1], in0=age_bc[:, q0:q1],
                                        scalar1=modage_t[:, kc:kc + 1],
                                        scalar2=nonneg[:, kc:kc + 1],
                                        op0=ALU.is_ge, op1=ALU.mult)

        # preload pw + residual x2 on sync queue (used later; queue is idle now)
        x2 = pool_x2.tile([128, QC, E], dt.float32, tag="x2")
        nc.sync.dma_start(x2[:], xh.rearrange("(c p) e -> p c e", p=128))
        pw_t = pool_pw.tile([128, FC, E], dt.bfloat16)
        nc.sync.dma_start(pw_t[:], pw)

        # ======== attention ========
        Yall8 = pool_yall.tile([128, EC, LQ], dt.float8e4, tag="Yall8")
        yall_dbg = None
        if "Yall" in dbg:
            yall_dbg = pool_yall.tile([128, EC, LQ], dt.bfloat16, tag="Ydbg")

        with tc.tile_pool(name="phD", bufs=nE + 1) as phD, \
             tc.tile_pool(name="phD_sm", bufs=3) as phD_sm, \
             tc.tile_pool(name="phD_rb", bufs=2) as phD_rb, \
             tc.tile_pool(name="phD_ps", bufs=2, space="PSUM") as phD_ps, \
             tc.tile_pool(name="phD_py", bufs=2, space="PSUM") as phD_py:
            warm_in = phD_sm.tile([1, 16], dt.float32, tag="warm")
            nc.vector.memset(warm_in[:], 1.0)
            warm_out = phD_sm.tile([DH, 16], dt.float32, tag="warmo")
            nc.gpsimd.partition_broadcast(warm_out[:], warm_in[:])
            pts = [[None] * nE for _ in range(H // 2)]
            pys = [None] * (H // 2)
            recs = [None] * (H // 2)

            def emit_S(hp, i):
                kc, q0, q1 = entries[i]
                w = LQ - q0
                ps = phD_ps.tile([128, 2, LQ], dt.float32, space="PSUM",
                                 tag="ps", name=f"ps{hp}_{i}")
                for hi in range(2):
                    nc.tensor.matmul(
                        ps[:, hi, 0:w],
                        KT[hi * DH:(hi + 1) * DH, hp, kc * 128:(kc + 1) * 128],
                        QT[hi * DH:(hi + 1) * DH, hp, q0:LQ],
                        start=True, stop=True, skip_group_check=True)
                pt = phD.tile([128, 2, LQ], dt.bfloat16, tag="pt",
                              name=f"pt{hp}_{i}")
                nc.scalar.activation(out=pt[:, :, 0:w], in_=ps[:, :, 0:w],
                                     func=AF.Exp, bias=0.0, scale=1.0)
                if q1 > q0:
                    mk = mask[:, kc, q0:q1]
                    mk2 = bass.AP(tensor=mk.tensor, offset=mk.offset,
                                  ap=[mk.ap[0], [0, 2], mk.ap[1]])
                    nc.vector.tensor_tensor(
                        out=pt[:, :, 0:q1 - q0], in0=pt[:, :, 0:q1 - q0],
                        in1=mk2, op=ALU.mult)
                pts[hp][i] = pt

            def emit_eps(hp):
                py = phD_py.tile([128, 2, LQ], dt.float32, space="PSUM",
                                 tag="py", name=f"py{hp}")
                pys[hp] = py
                for hi in range(2):
                    nc.tensor.matmul(py[0:DH + 1, hi, :], epsc[:], ones1p[:],
                                     start=True, stop=False,
                                     skip_group_check=True)

            def emit_PV(hp, i):
                kc, q0, q1 = entries[i]
                w = LQ - q0
                py = pys[hp]
                for hi in range(2):
                    h = 2 * hp + hi
                    nc.tensor.matmul(
                        py[0:DH + 1, hi, q0:LQ],
                        Vaug[:, kc, h * (DH + 1):(h + 1) * (DH + 1)],
                        pts[hp][i][:, hi, 0:w],
                        start=False, stop=(i == nE - 1), skip_group_check=True)

            def start_div(hp):
                den = phD_sm.tile([1, 2, LQ], dt.float32, tag="den")
                nc.vector.tensor_copy(den[:], pys[hp][DH:DH + 1, :, :])
                rec = phD_sm.tile([1, 2, LQ], dt.float32, tag="rec")
                nc.vector.reciprocal_approx_fast(out=rec[:], in_=den[:])
                recs[hp] = rec

            def finish_div(hp):
                recb = phD_rb.tile([DH, 2, LQ], dt.float32, tag="recb")
                nc.gpsimd.partition_broadcast(recb[:], recs[hp][:])
                for hi in range(2):
                    nc.vector.tensor_tensor(
                        out=Yall8[hi * DH:(hi + 1) * DH, hp, :],
                        in0=pys[hp][0:DH, hi, :], in1=recb[:, hi, :],
                        op=ALU.mult)
                    if yall_dbg is not None:
                        nc.vector.tensor_tensor(
                            out=yall_dbg[hi * DH:(hi + 1) * DH, hp, :],
                            in0=pys[hp][0:DH, hi, :], in1=recb[:, hi, :],
                            op=ALU.mult)

            for hp in range(H // 2):
                for i in range(nE):
                    emit_S(hp, i)
                    if hp > 0:
                        if i == 0:
                            emit_eps(hp - 1)
                        emit_PV(hp - 1, i)
                        if i == min(2, nE - 1) and hp > 1:
                            finish_div(hp - 2)
                if hp > 0:
                    start_div(hp - 1)
            last = H // 2 - 1
            for i in range(nE):
                if i == 0:
                    emit_eps(last)
                emit_PV(last, i)
                if i == min(2, nE - 1):
                    finish_div(last - 1)
            start_div(last)
            finish_div(last)
        if "Yall" in dbg:
            nc.sync.dma_start(dbg["Yall"], yall_dbg[:])
        pool_att.release()

        # ======== c-proj + residual + LN2 ========
        h1nT = pool_mlp.tile([128, EC, LQ], dt.float8e4, tag="h1nT")
        with tc.tile_pool(name="phE", bufs=2) as phE, \
             tc.tile_pool(name="phE_ps", bufs=3, space="PSUM") as phE_ps, \
             tc.tile_pool(name="phE_ln", bufs=3, space="PSUM") as phE_ln:
            ln2 = make_ln(x2[:], QC, ln2w_c, ln2b_c, ln_ident[2], h1nT,
                          phE, phE_ln, "l2", xhat_scalar=True)
            for qc in range(QC):
                for hf in range(2):
                    pc = phE_ps.tile([128, 384], dt.float32, space="PSUM",
                                     tag="pc")
                    for j in range(3):
                        nc.tensor.matmul(
                            pc[:], Yall8[:, 2 * j:2 * j + 2, qc * 128:(qc + 1) * 128],
                            cw8_t[:, j, :, hf * 384:(hf + 1) * 384],
                            start=(j == 0), stop=(j == 2), perf_mode=PM.DoubleRow)
                    sl = slice(hf * 384, (hf + 1) * 384)
                    tmp = phE.tile([128, 384], dt.float32, tag="ctmp")
                    if zb[1]:
                        nc.scalar.activation(out=tmp[:], in_=pc[:],
                                             func=AF.Copy, bias=0.0,
                                             scale=1.0 / WS)
                    else:
                        nc.vector.scalar_tensor_tensor(
                            out=tmp[:], in0=pc[:], scalar=1.0 / WS,
                            in1=cb_r[:, sl], op0=ALU.mult, op1=ALU.add)
                    nc.vector.tensor_add(out=x2[:, qc, sl], in0=x2[:, qc, sl],
                                         in1=tmp[:])
                ln2(qc)
        pool_yall.release()
        if "x2" in dbg:
            nc.sync.dma_start(dbg["x2"], x2[:])

        # ======== MLP ========
        hT = pool_mlp.tile([128, FC, LQ], dt.bfloat16, tag="hT")
        with tc.tile_pool(name="phF", bufs=2) as phF, \
             tc.tile_pool(name="phF_ps", bufs=3, space="PSUM") as phF_ps:
            for fg in range(FC // 8):
                fw = phF.tile([128, 8, 3, 2, 128], dt.float8e4, tag="fw")
                nc.sync.dma_start(fw[:], fcw[:, fg])
                for fi in range(8):
                    f = fg * 8 + fi
                    ph = phF_ps.tile([128, LQ], dt.float32, space="PSUM",
                                     tag="ph")
                    for j in range(3):
                        nc.tensor.matmul(ph[:], fw[:, fi, j, :, :],
                                         h1nT[:, 2 * j:2 * j + 2, :],
                                         start=(j == 0), stop=(j == 2),
                                         perf_mode=PM.DoubleRow)
                    nc.scalar.activation(out=hT[:, f, :], in_=ph[:],
                                         func=AF.Gelu,
                                         bias=fcb_c[:, f:f + 1], scale=1.0 / WS)

        with tc.tile_pool(name="phG_out", bufs=2) as phG_out, \
             tc.tile_pool(name="phG_ps", bufs=4, space="PSUM") as phG_ps:
            for qc in range(QC):
                pps = [phG_ps.tile([128, 384], dt.float32, space="PSUM",
                                   tag="pp", name=f"pp{qc}_{hf}")
                       for hf in range(2)]
                for f in range(FC):
                    for hf in range(2):
                        nc.tensor.matmul(
                            pps[hf][:], hT[:, f, qc * 128:(qc + 1) * 128],
                            pw_t[:, f, hf * 384:(hf + 1) * 384],
                            start=(f == 0), stop=(f == FC - 1),
                            skip_group_check=True)
                ot = phG_out.tile([128, E], dt.float32, tag="ot")
                for hf in range(2):
                    sl = slice(hf * 384, (hf + 1) * 384)
                    if zb[2]:
                        nc.scalar.activation(out=ot[:, sl], in_=pps[hf][:],
                                             func=AF.Copy, bias=0.0, scale=1.0)
                    else:
                        nc.vector.scalar_tensor_tensor(
                            out=ot[:, sl], in0=pps[hf][:], scalar=1.0,
                            in1=pb_r[:, sl], op0=ALU.mult, op1=ALU.add)
                nc.vector.tensor_add(out=ot[:], in0=ot[:], in1=x2[:, qc, :])
                q = nc.sync if qc % 2 == 0 else nc.scalar
                q.dma_start(
                    out.rearrange("(c p) e -> p c e", p=128)[:, qc, :], ot[:])

        pool_w.release()
        pool_x2.release()
        pool_mlp.release()
        pool_pw.release()
        singles.release()

    nc.compile()
    return nc


def _pairs_img(wT, scale):
    """wT [E_in, D] -> [128, E_in//256, 2, D] fp8 chunk-pair image."""
    Ei, D = wT.shape
    w = (np.asarray(wT, np.float32) * scale).reshape(Ei // 128, 128, D)
    # [j, i] -> chunk 2j+i
    return np.ascontiguousarray(
        w.reshape(Ei // 256, 2, 128, D).transpose(2, 0, 1, 3)).astype(F8)


def _host_prep(x, age, mod_idx, mod_age, mod2_emb, mod3_emb,
               ln0_w, ln0_b, ln1_w, ln1_b, ln2_w, ln2_b,
               q_w, q_b, k_w, k_b, v_w, v_b, c_w, c_b,
               fc_w, fc_b, proj_w, proj_b):
    f32 = np.float32
    x = np.asarray(x, f32); age = np.asarray(age, f32)
    mod_idx = np.asarray(mod_idx); mod_age = np.asarray(mod_age, f32)
    mod2_emb = np.asarray(mod2_emb, f32); mod3_emb = np.asarray(mod3_emb, f32)

    qw8 = _pairs_img(np.asarray(q_w, f32).T, WS)
    kw8 = _pairs_img(np.asarray(k_w, f32).T, WS)
    vw8 = _pairs_img(np.asarray(v_w, f32).T, WS)
    cw8 = _pairs_img(np.asarray(c_w, f32).T, WS)
    # fcw [128, 3, 8, EC, 128]: [p, fg, fi, c, d] = fc_w.T[c*128+p, (fg*8+fi)*128+d]
    fcwT = np.asarray(fc_w, f32).T * WS  # [E, F]
    # [p, fg, fi, j, i, d] = fcwT[(2j+i)*128+p, (fg*8+fi)*128+d]
    fcw = np.ascontiguousarray(
        fcwT.reshape(3, 2, 128, FC // 8, 8, 128).transpose(2, 3, 4, 0, 1, 5)
    ).astype(F8)
    pwT = np.asarray(proj_w, f32).T  # [F, E]
    pw = np.ascontiguousarray(
        pwT.reshape(FC, 128, E).transpose(1, 0, 2)).astype(BF)

    scale = np.float32(DH) ** -0.5
    qb_s = np.asarray(q_b, f32) * scale

    def col6(a):  # [E] -> [128, 6]
        return np.asarray(a, f32).reshape(6, 128).T

    def col24(a):  # [F] -> [128, 24]
        return np.asarray(a, f32).reshape(24, 128).T

    blob = np.concatenate([
        col6(qb_s), col6(k_b), col24(fc_b),
        col6(ln0_w), col6(ln0_b), col6(ln1_w), col6(ln1_b),
        col6(ln2_w), col6(ln2_b),
        np.broadcast_to(np.asarray(v_b, f32), (128, E)),
        np.broadcast_to(np.asarray(c_b, f32), (128, E)),
        np.broadcast_to(np.asarray(proj_b, f32), (128, E)),
    ], axis=1).astype(f32)
    blob = np.ascontiguousarray(blob)

    lnp = [np.asarray(a, f32) for a in
           (ln0_w, ln0_b, ln1_w, ln1_b, ln2_w, ln2_b)]
    ln_ident = tuple(
        bool(np.all(lnp[2 * i] == 1.0) and np.all(lnp[2 * i + 1] == 0.0))
        for i in range(3))
    zb = (bool(np.all(np.asarray(v_b) == 0.0)),
          bool(np.all(np.asarray(c_b) == 0.0)),
          bool(np.all(np.asarray(proj_b) == 0.0)))

    # fold attention scale into qw8 already? No: fold into QT copy via
    # 1/(WS*8): qw8 holds q_w*WS, ACT scale divides by WS*8.

    qrows = {0: np.arange(0, 2 * LQ, 2), 1: np.arange(1, 2 * LQ, 2)}

    shared = dict(qw8=qw8, kw8=kw8, vw8=vw8, cw8=cw8, fcw=fcw, pw=pw,
                  blob=blob)

    in_maps = []
    q0s = np.full(KC, LQ, dtype=np.int64)
    q1s = np.zeros(KC, dtype=np.int64)
    any_neg = np.zeros(KC, dtype=bool)
    for core in range(N_CORES):
        b, g = core // 2, core % 2
        rows = qrows[g]
        order = np.argsort(mod_age[b], kind="stable")
        s_idx = np.asarray(mod_idx[b])[order]
        m2 = s_idx == 2
        m3 = s_idx == 3
        occ2 = np.clip(np.cumsum(m2) - 1, 0, HALF - 1)
        occ3 = np.clip(np.cumsum(m3) - 1, 0, HALF - 1)
        gi = np.full(LK, 2 * HALF, dtype=np.int32)
        gi[m2] = occ2[m2]
        gi[m3] = HALF + occ3[m3]
        embcat = np.concatenate([
            mod2_emb[b * HALF:(b + 1) * HALF],
            mod3_emb[b * HALF:(b + 1) * HALF],
            np.zeros((1, E), f32)], axis=0).astype(BF)
        agec = np.ascontiguousarray(age[b][rows])
        ma = mod_age[b]
        for kc in range(KC):
            ch = ma[kc * 128:(kc + 1) * 128]
            neg = ch < 0.0
            any_neg[kc] |= bool(neg.any())
            pos = ch[~neg]
            if pos.size == 0:
                continue  # chunk fully padded -> dead
            lo, hi = pos.min(), pos.max()
            q0 = int(np.searchsorted(agec, lo, side="left"))
            q1 = int(np.searchsorted(agec, hi, side="left"))
            q0s[kc] = min(q0s[kc], q0)
            q1s[kc] = max(q1s[kc], q1)
        in_maps.append(dict(
            xh=np.ascontiguousarray(x[b][rows]),
            embcat=embcat, gidx=gi, ageh=agec,
            modage=np.ascontiguousarray(ma), **shared))

    entries = []
    for kc in range(KC):
        if q0s[kc] >= LQ:
            continue  # never live on any core
        q1 = LQ if any_neg[kc] else min(int(q1s[kc]), LQ)
        entries.append((kc, int(q0s[kc]), q1))
    entries = tuple(entries)
    return in_maps, (entries, ln_ident, zb), qrows


def _run(inputs, trace):
    in_maps, key, qrows = _host_prep(**inputs)
    if key not in _prog_cache:
        _prog_cache[key] = _build_program(*key)
    nc = _prog_cache[key]
    res = run_bass_kernel_spmd(nc, in_maps, core_ids=list(range(N_CORES)),
                               trace=trace)
    out = np.empty((B, 2 * LQ, E), dtype=np.float32)
    for core in range(N_CORES):
        b, g = core // 2, core % 2
        out[b, qrows[g]] = res.results[core]["out"]
    return out, res


def kernel(**inputs):
    return _run(inputs, trace=False)[0]


def run_traced(**inputs):
    return _run(inputs, trace=True)
